# revision 56
# baseline (speedup 1.0000x reference)
"""Bass/Trainium2 kernel for nn_DSQGAttentionD41J16D (sparse offset attention).

Sharding: 16 heads over 8 cores -> 2 heads per core (SPMD). Host lays inputs
out transposed per core as [dh = h*64+d (128 partitions), 1024-pad + n] bf16,
so every offset-shift k[n-d_i] / v[n-d_i] is a free-dim slice on chip.

Window-pipelined schedule (window c ~= products of chunk c on DVE):
  DVE : [prods c0][prods c1][pvD c0][prods c2][pvD c1][prods c3][pvD c2][pvD c3]
  Pool: [pad memsets][5 early c3 products][pvP c0][pvP c1][pvP c2][pvP c3]
  PE  : [warm][sel c0][sel c1][acc c0 + psl c0][sel c2][acc c1 + psl c1] ...
  ACT : [q0/pb loads][exp c0][exp c1][outcopy c0 + out dma c0][exp c2] ...
  SP  : [k/q/v/blob loads][bcast c0 (pool-first)][bcast c1] ...

Scores: pss[(i,h), n] = sel-matmul partition-reduce of DVE products
q*k[n-d_i] plus an se-matmul, -1e30 validity mask for chunk 0; p = exp(
0.125*pss + pos_bias) on ACT (bias port). Denominator l via ones-matmuls
(PE) shipped to host; host divides and zeroes the n=0 row.
PV: p rows broadcast across the 64 d-partitions by repeat-read DMAs (SP
queue), tmp_i = p_bc * vT[n-d_i] split DVE/Pool, accumulated on PE via
identity matmuls into PSUM fp32.
"""

import os
import sys

sys.path.insert(0, "/opt/trn_rl_repo")

import numpy as np
import ml_dtypes

ALL_OFFSETS = [1, 3, 4, 13, 15, 21, 23, 28, 48, 64, 96, 192, 384, 512, 768, 1024]
N = 4096
HD = 64
P = 128
PAD = 1024
NT = PAD + N
NOFF = 16
C = 1024          # chunk width
NCH = N // C      # 4 chunks
NEG = -1.0e30

BF16 = ml_dtypes.bfloat16
FP8 = ml_dtypes.float8_e4m3fn
# subtracted from pos_bias host-side so exp() outputs stay in fp8-e4m3 range
# for the Pool fp8 path (cancels in the softmax ratio; l is consistent).
PB_BIAS = -2.0

# PV multiply jobs routed to the GPSIMD (Pool) engine, as offset PAIRS whose
# fp8 tmp tiles are written column-interleaved so one DoubleRow matmul
# accumulates both (offsets must have d < 512 so both halves stay valid).
POOL_PV = (
    {(c, i) for c in range(3) for i in (1, 4, 7, 10)} | {(3, 1), (3, 4)}
)
# Score products computed on Pool up front (chunk-1 pieces of k/q load right
# after chunk 0, so Pool streams products while waiting for broadcasts).
POOL_PRODS = {1: (0, 2, 5, 8, 11, 14), 2: (0, 5), 3: (0, 5)}

_CACHE = {}
TRACE = os.environ.get("BASS_KERNEL_TRACE", "0") == "1"
LAST_RESULTS = [None]

# const blob column layout
SEL0 = 0            # sel: 16 offsets x 32 cols
SET0 = 512          # seT: 32 cols
ID0 = 544           # ident: 128 cols
MKT0 = 672          # maskT: 32 cols (rows 0:16)
MK0 = 704           # mask01: 1024 cols (rows 0:16)
ONL0 = 1728         # onesl: 4 x 8 cols (rows 0:32)
BLOBW = 1760

# fp8 const blob: double identity [I | I] for DoubleRow accumulate
ID8D0 = 0
BLOB8W = 256


def _build_blob(scale_embed_np):
    blob = np.zeros((P, BLOBW), dtype=BF16)
    for h in range(2):
        for d in range(HD):
            p = 64 * h + d
            for i in range(NOFF):
                blob[p, SEL0 + 32 * i + 2 * i + h] = 1.0
                blob[p, SET0 + 2 * i + h] = BF16(scale_embed_np[i, d])
    blob[:, ID0 : ID0 + P] = np.eye(P, dtype=BF16)
    for j in range(NOFF):
        blob[j, MKT0 + 2 * j] = NEG
        blob[j, MKT0 + 2 * j + 1] = NEG
        blob[j, MK0 : MK0 + ALL_OFFSETS[j]] = 1.0
    for hf in range(4):
        for i in range(NOFF):
            for h in range(2):
                blob[2 * i + h, ONL0 + 8 * hf + 2 * hf + h] = 1.0
    return blob


def _build_blob8():
    blob8 = np.zeros((P, BLOB8W), dtype=FP8)
    blob8[:, ID8D0 : ID8D0 + P] = np.eye(P, dtype=FP8)
    blob8[:, ID8D0 + P : ID8D0 + 2 * P] = np.eye(P, dtype=FP8)
    return blob8


def _build(scale_embed_np):
    import concourse.bass as bass
    import concourse.mybir as mybir
    import concourse.tile as tile
    from concourse import bacc

    import bass_rust

    fp32 = mybir.dt.float32
    bf16 = mybir.dt.bfloat16
    fp8 = mybir.dt.float8e4
    DROW = bass_rust.MatmulPerfMode.DoubleRow
    MULT = mybir.AluOpType.mult
    EXP = mybir.ActivationFunctionType.Exp
    COPY = mybir.ActivationFunctionType.Copy

    nc = bacc.Bacc()

    qT_in = nc.dram_tensor("qT_in", [P, NT], bf16, kind="ExternalInput")
    kT_in = nc.dram_tensor("kT_in", [P, NT], bf16, kind="ExternalInput")
    vT_in = nc.dram_tensor("vT_in", [P, NT], bf16, kind="ExternalInput")
    pb_in = nc.dram_tensor("pb_in", [2 * NOFF, 1], bf16, kind="ExternalInput")
    oT_out = nc.dram_tensor("oT_out", [P, N], bf16, kind="ExternalOutput")
    l_out = nc.dram_tensor("l_out", [8, 1024], fp32, kind="ExternalOutput")

    blob_c = nc.inline_tensor(_build_blob(scale_embed_np), name="blob_c")

    with tile.TileContext(nc) as tc:
        consts = tc.alloc_tile_pool(name="consts", bufs=1)
        big = tc.alloc_tile_pool(name="big", bufs=1)
        ps_s = tc.alloc_tile_pool(name="ps_s", bufs=2, space="PSUM")
        ps_l = tc.alloc_tile_pool(name="ps_l", bufs=2, space="PSUM")
        ps_a = tc.alloc_tile_pool(name="ps_a", bufs=4, space="PSUM")
        work = tc.alloc_tile_pool(name="work", bufs=18)
        workh = tc.alloc_tile_pool(name="workh", bufs=8)
        tmps = tc.alloc_tile_pool(name="tmps", bufs=12)
        tmpsh = tc.alloc_tile_pool(name="tmpsh", bufs=6)
        bcast = tc.alloc_tile_pool(name="bcast", bufs=24)
        bcasth = tc.alloc_tile_pool(name="bcasth", bufs=28)

        qT = big.tile([P, NT], bf16)
        kT = big.tile([P, NT], bf16)
        vT = big.tile([P, NT], bf16)
        p_sb = big.tile([2 * NOFF, N], bf16)
        out_sb = big.tile([P, N], bf16)
        l_sb = big.tile([8, 1024], fp32)
        blob = consts.tile([P, BLOBW], bf16)
        pb_sb = consts.tile([2 * NOFF, 1], bf16)

        # ---- loads: q0 + k0 halves first (the first chunk-0 products d=512,
        # 768 need only q0 + pad + the first k0 half), then k1/q1 (early Pool
        # products), blob (PE warm/sel), the rest, then v (PV phase).
        nc.scalar.dma_start(out=qT[:, PAD : PAD + 512], in_=qT_in[:, PAD : PAD + 512])
        nc.sync.dma_start(out=kT[:, PAD : PAD + 512], in_=kT_in[:, PAD : PAD + 512])
        nc.gpsimd.memset(kT[:, 0:PAD], 0.0)
        nc.gpsimd.memset(vT[:, 0:PAD], 0.0)
        nc.sync.dma_start(out=blob, in_=blob_c[:, :])
        nc.scalar.dma_start(out=qT[:, PAD + 512 : PAD + C], in_=qT_in[:, PAD + 512 : PAD + C])
        nc.sync.dma_start(out=kT[:, PAD + 512 : PAD + C], in_=kT_in[:, PAD + 512 : PAD + C])
        nc.sync.dma_start(out=kT[:, PAD + C : PAD + 2 * C], in_=kT_in[:, PAD + C : PAD + 2 * C])
        nc.sync.dma_start(out=qT[:, PAD + C : PAD + 2 * C], in_=qT_in[:, PAD + C : PAD + 2 * C])
        nc.scalar.dma_start(out=pb_sb, in_=pb_in[:, :])
        nc.sync.dma_start(out=kT[:, PAD + 2 * C : NT], in_=kT_in[:, PAD + 2 * C : NT])
        nc.sync.dma_start(out=qT[:, PAD + 2 * C : NT], in_=qT_in[:, PAD + 2 * C : NT])
        nc.sync.dma_start(out=vT[:, PAD : NT], in_=vT_in[:, PAD : NT])

        # ---- blob slices ----
        def sel_w(i):
            return blob[:, SEL0 + 32 * i : SEL0 + 32 * i + 32]

        seT_w = blob[:, SET0 : SET0 + 32]
        ident_w = blob[:, ID0 : ID0 + P]
        maskT_w = blob[0:NOFF, MKT0 : MKT0 + 32]

        def mask01_x(g):
            return blob[0:NOFF, MK0 + 512 * g : MK0 + 512 * g + 512]

        def onesl_w(hf):
            return blob[0 : 2 * NOFF, ONL0 + 8 * hf : ONL0 + 8 * hf + 8]

        # single warm-up matmul: consumes the blob DMA semaphore wait early
        warm = ps_s.tile([P, 2], fp32, tag="pss", name="warm")
        nc.tensor.matmul(warm[0:32, 0:2], sel_w(0), ident_w[:, 0:2],
                         start=True, stop=True)

        # validity skip rule: offset i fully invalid in 512-chunk g iff
        # delta_i >= 512*(g+1)
        def sel_skip(g, i):
            return ALL_OFFSETS[i] >= 512 * (g + 1)

        def vstart(c, i):
            """512-aligned start of the valid region of (chunk c, offset i);
            >= C means fully skippable."""
            v = max(0, ALL_OFFSETS[i] - C * c)
            return (v // 512) * 512

        prod_tiles = {}

        def emit_product(c, i, eng):
            b = C * c
            d = ALL_OFFSETS[i]
            pr = work.tile([P, C], bf16, tag="prod", name=f"prod_{c}_{i}")
            eng.tensor_tensor(
                out=pr,
                in0=qT[:, PAD + b : PAD + b + C],
                in1=kT[:, PAD + b - d : PAD + b - d + C],
                op=MULT,
            )
            prod_tiles[(c, i)] = pr

        def emit_product_half(c, i, g):
            # 512-wide product for (chunk c, offset i, half g): chunk 0 runs
            # entirely in halves so the g0 score chain (and its exp +
            # broadcasts) starts as soon as the first q/k half-loads land.
            b = C * c + 512 * g
            d = ALL_OFFSETS[i]
            pr = workh.tile([P, 512], bf16, tag="prodh", name=f"prod_{c}_{i}_{g}")
            nc.vector.tensor_tensor(
                out=pr,
                in0=qT[:, PAD + b : PAD + b + 512],
                in1=kT[:, PAD + b - d : PAD + b - d + 512],
                op=MULT,
            )
            prod_tiles[(c, i, g)] = pr

        # ---- early Pool products while Pool waits for bcasts ----
        for i in POOL_PRODS[1]:
            emit_product(1, i, nc.gpsimd)

        psl8 = [None, None]

        def emit_psl(c):
            """Denominator row-sum matmuls for chunk c's two 512-chunks."""
            pair = c // 2
            for gl in range(2):
                g = 2 * c + gl
                s0 = 512 * g
                hf = g - 4 * pair
                if hf == 0:
                    psl8[pair] = ps_l.tile(
                        [8, 512], fp32, tag="psl", name=f"psl_{pair}"
                    )
                nc.tensor.matmul(
                    psl8[pair], onesl_w(hf), p_sb[:, s0 : s0 + 512],
                    start=(hf == 0), stop=(hf == 3), skip_group_check=True,
                )

        def emit_scores0():
            # chunk 0, one 512-half at a time: [seT, products+sel, mask, exp]
            # per half, so exp(g0) fires ~9us in and broadcasts start early.
            pss = {}
            for g in (0, 1):
                # g0: offsets d<512 need only the first q/k half-loads;
                # g1: d=512/768 first (need only k0a+q0b), then the rest
                iord = (
                    [i for i in range(13)] if g == 0
                    else [13, 14] + [i for i in range(13)]
                )
                pss[g] = ps_s.tile([2 * NOFF, 512], fp32, tag="pss",
                                   name=f"pss_{g}")
                nc.tensor.matmul(
                    pss[g], seT_w, qT[:, PAD + 512 * g : PAD + 512 * g + 512],
                    start=True, stop=False, skip_group_check=True,
                )
                for i in iord:
                    emit_product_half(0, i, g)
                    nc.tensor.matmul(
                        pss[g], sel_w(i), prod_tiles[(0, i, g)],
                        start=False, stop=False, skip_group_check=True,
                    )
                nc.tensor.matmul(
                    pss[g], maskT_w, mask01_x(g),
                    start=False, stop=True, skip_group_check=True,
                )
                nc.scalar.activation(
                    out=p_sb[:, 512 * g : 512 * g + 512], in_=pss[g],
                    func=EXP, scale=0.125, bias=pb_sb[:, 0:1],
                )

        def emit_scores(c):
            b = C * c
            g0, g1 = 2 * c, 2 * c + 1
            pss = {}
            for gl, g in ((0, g0), (1, g1)):
                pss[gl] = ps_s.tile(
                    [2 * NOFF, 512], fp32, tag="pss", name=f"pss_{g}"
                )
                nc.tensor.matmul(
                    pss[gl], seT_w,
                    qT[:, PAD + 512 * g : PAD + 512 * g + 512],
                    start=True, stop=False, skip_group_check=True,
                )
            # chunk 0: products d=512/768 first — they need only the first
            # half of k0, so DVE starts ~1us earlier
            iorder = (
                [13, 14] + [i for i in range(NOFF) if i not in (13, 14)]
                if c == 0 else list(range(NOFF))
            )
            valid0 = [i for i in iorder if not sel_skip(g0, i)]
            valid1 = [i for i in iorder if not sel_skip(g1, i)]
            masked = g0 < 2
            for i in iorder:
                in0, in1 = i in valid0, i in valid1
                if not (in0 or in1):
                    continue
                if (c, i) not in prod_tiles:
                    emit_product(c, i, nc.vector)
                pr = prod_tiles[(c, i)]
                if in0:
                    nc.tensor.matmul(
                        pss[0], sel_w(i), pr[:, 0:512],
                        start=False,
                        stop=(not masked) and i == valid0[-1],
                        skip_group_check=True,
                    )
                if in1:
                    nc.tensor.matmul(
                        pss[1], sel_w(i), pr[:, 512:1024],
                        start=False,
                        stop=(not masked) and i == valid1[-1],
                        skip_group_check=True,
                    )
            if masked:
                for gl, g in ((0, g0), (1, g1)):
                    nc.tensor.matmul(
                        pss[gl], maskT_w, mask01_x(g),
                        start=False, stop=True, skip_group_check=True,
                    )
            for gl, g in ((0, g0), (1, g1)):
                nc.scalar.activation(
                    out=p_sb[:, 512 * g : 512 * g + 512], in_=pss[gl],
                    func=EXP, scale=0.125, bias=pb_sb[:, 0:1],
                )

        def pv_order(c):
            """PV offsets of chunk c: pool-share first, then DVE-share."""
            alive = [i for i in range(NOFF) if vstart(c, i) < C]
            return (
                [i for i in alive if (c, i) in POOL_PV]
                + [i for i in alive if (c, i) not in POOL_PV]
            )

        def pv_halves0():
            """Chunk-0 PV jobs as (i, g) halves: pool first, g0 before g1.
            Half g is valid for offset i iff d < 512*(g+1) (i<15 always has
            some valid half; g0 needs d<512)."""
            out = []
            for pool_pass in (True, False):
                for g in (0, 1):
                    for i in range(15):
                        if g == 0 and ALL_OFFSETS[i] >= 512:
                            continue
                        if ((0, i) in POOL_PV) == pool_pass:
                            out.append((i, g))
            return out

        pbc_tiles = {}

        def emit_pbc(c):
            b = C * c
            for i in pv_order(c):
                v5 = vstart(c, i)
                w = C - v5
                rows = p_sb[2 * i : 2 * i + 2, b + v5 : b + v5 + w]
                dst = bcast.tile(
                    [P, w], bf16, tag="pbc", name=f"pbc_{c}_{i}",
                )
                rep = bass.AP(
                    tensor=rows.tensor,
                    offset=rows.offset,
                    ap=[list(rows.ap[0]), [0, HD], [1, w]],
                )
                nc.sync.dma_start(out=dst, in_=rep)
                pbc_tiles[(c, i)] = dst

        def emit_pbc0():
            # chunk-0 broadcasts per 512-half, so they start right after
            # exp(g0) instead of waiting for the whole chunk
            for i, g in pv_halves0():
                rows = p_sb[2 * i : 2 * i + 2, 512 * g : 512 * g + 512]
                dst = bcasth.tile([P, 512], bf16, tag="pbch",
                                 name=f"pbc_0_{i}_{g}")
                rep = bass.AP(
                    tensor=rows.tensor,
                    offset=rows.offset,
                    ap=[list(rows.ap[0]), [0, HD], [1, 512]],
                )
                nc.sync.dma_start(out=dst, in_=rep)
                pbc_tiles[(0, i, g)] = dst

        acc = {}

        def emit_pv0():
            jobs = pv_halves0()
            npass = [len([1 for i, g in jobs if g == gl]) for gl in range(2)]
            started = [False, False]
            for gl in range(2):
                acc[gl] = ps_a.tile([P, 512], fp32, tag="acc", name=f"acc_{gl}")
            for i, g in jobs:
                d = ALL_OFFSETS[i]
                tmp = tmpsh.tile([P, 512], bf16, tag="tmph",
                                name=f"tmp_0_{i}_{g}")
                eng = nc.gpsimd if (0, i) in POOL_PV else nc.vector
                eng.tensor_tensor(
                    out=tmp,
                    in0=pbc_tiles[(0, i, g)],
                    in1=vT[:, PAD + 512 * g - d : PAD + 512 * g - d + 512],
                    op=MULT,
                )
                npass[g] -= 1
                nc.tensor.matmul(
                    acc[g], ident_w, tmp,
                    start=(not started[g]), stop=(npass[g] == 0),
                    skip_group_check=True,
                )
                started[g] = True

        def emit_pv(c):
            """PV multiplies: Pool-share first (its bcasts were issued
            first), then DVE-share; identity-matmul accumulation follows
            each multiply on PE."""
            b = C * c
            order = pv_order(c)
            valid_g = [
                [i for i in order if vstart(c, i) <= 512 * gl] for gl in range(2)
            ]
            for gl in range(2):
                acc[2 * c + gl] = ps_a.tile(
                    [P, 512], fp32, tag="acc", name=f"acc_{2 * c + gl}"
                )
            started = [False, False]
            remaining = [len(valid_g[0]), len(valid_g[1])]
            for i in order:
                v5 = vstart(c, i)
                d = ALL_OFFSETS[i]
                w = C - v5
                tmp = tmps.tile([P, w], bf16, tag="tmp", name=f"tmp_{c}_{i}")
                eng = nc.gpsimd if (c, i) in POOL_PV else nc.vector
                eng.tensor_tensor(
                    out=tmp,
                    in0=pbc_tiles[(c, i)],
                    in1=vT[:, PAD + b + v5 - d : PAD + b + v5 - d + w],
                    op=MULT,
                )
                for gl in range(2):
                    if i not in valid_g[gl]:
                        continue
                    s0l = 512 * gl
                    remaining[gl] -= 1
                    nc.tensor.matmul(
                        acc[2 * c + gl], ident_w,
                        tmp[:, s0l - v5 : s0l - v5 + 512],
                        start=(not started[gl]),
                        stop=(remaining[gl] == 0),
                        skip_group_check=True,
                    )
                    started[gl] = True

        def emit_out(c):
            # per-half copy + DMA so the first half's output ships while the
            # second half is still copying (shortens the tail)
            for gl in range(2):
                g = 2 * c + gl
                s0 = 512 * g
                nc.scalar.activation(
                    out=out_sb[:, s0 : s0 + 512], in_=acc[g], func=COPY
                )
                nc.scalar.dma_start(
                    out=oT_out[:, s0 : s0 + 512], in_=out_sb[:, s0 : s0 + 512]
                )

        def emit_lcopy(pair):
            nc.scalar.activation(
                out=l_sb[:, 512 * pair : 512 * pair + 512],
                in_=psl8[pair], func=COPY,
            )

        # ---- pipelined emission: DVE runs products c0,c1,c2, squeezes in
        # pv(c0) while bcasts for c1/c2 land, then products c3 and the
        # remaining pv windows. Pool front-loads products (c1,c2,c3) then
        # drains its pv shares. ----
        emit_scores0()
        emit_pbc0()
        for cc in (2, 3):
            for i in POOL_PRODS[cc]:
                emit_product(cc, i, nc.gpsimd)
        emit_scores(1)
        emit_pbc(1)
        emit_scores(2)
        emit_pbc(2)
        emit_psl(0)
        emit_pv0()
        emit_out(0)
        emit_scores(3)
        emit_pbc(3)
        emit_psl(1)
        emit_lcopy(0)                    # psl pair 0 complete after psl(c1)
        emit_pv(1)
        emit_out(1)
        emit_psl(2)
        emit_pv(2)
        emit_out(2)
        emit_psl(3)
        emit_lcopy(1)
        emit_pv(3)
        emit_out(3)
        nc.scalar.dma_start(out=l_out[:, :], in_=l_sb)

        bcasth.release()
        bcast.release()
        tmpsh.release()
        tmps.release()
        workh.release()
        work.release()
        ps_a.release()
        ps_l.release()
        ps_s.release()
        big.release()
        consts.release()

    nc.compile()
    return nc


def _prep_inputs(q, k, v, pos_bias):
    """Host-side sharding + layout prep: per core, heads (2c, 2c+1) packed as
    128 partitions (h*64+d), transposed to [dh, pad+n] bf16."""
    def to_T(x):
        xt = np.ascontiguousarray(x[0].transpose(0, 2, 1)).astype(BF16)
        xt = xt.reshape(8, P, N)
        return np.concatenate([np.zeros((8, P, PAD), dtype=BF16), xt], axis=2)

    qT = to_T(q)
    kT = to_T(k)
    vT = to_T(v)

    in_maps = []
    for c in range(8):
        pb = np.zeros((2 * NOFF, 1), dtype=np.float32)
        for i in range(NOFF):
            for hh in range(2):
                pb[2 * i + hh, 0] = pos_bias[i, 2 * c + hh] + PB_BIAS
        in_maps.append(
            {
                "qT_in": qT[c],
                "kT_in": kT[c],
                "vT_in": vT[c],
                "pb_in": pb.astype(BF16),
            }
        )
    return in_maps


def kernel(q, k, v, pos_bias, scale_embed):
    from concourse.bass_utils import run_bass_kernel_spmd

    q = np.asarray(q)
    k = np.asarray(k)
    v = np.asarray(v)
    pos_bias = np.asarray(pos_bias)
    scale_embed = np.asarray(scale_embed)
    assert q.shape == (1, 16, N, HD)

    key = scale_embed.tobytes()
    if key not in _CACHE:
        _CACHE.clear()
        _CACHE[key] = _build(scale_embed)
    nc = _CACHE[key]

    in_maps = _prep_inputs(q, k, v, pos_bias)
    res = run_bass_kernel_spmd(nc, in_maps, core_ids=list(range(8)), trace=TRACE)
    LAST_RESULTS[0] = res
    out = np.zeros((1, 16, N, HD), dtype=np.float32)
    for c in range(8):
        oT = res.results[c]["oT_out"]          # [128, N] bf16 (unnormalized)
        lv = res.results[c]["l_out"]           # [8, 1024] fp32
        # l[h, n]: n = 2048*pair + 512*hf + j -> l_out[2*hf+h, 512*pair+j]
        l = np.zeros((2, N), dtype=np.float32)
        for h in range(2):
            for pair in range(2):
                for hf in range(4):
                    n0 = 2048 * pair + 512 * hf
                    l[h, n0 : n0 + 512] = lv[2 * hf + h, 512 * pair : 512 * pair + 512]
        l = np.where(l > 0.0, l, 1.0)
        o = oT.astype(np.float32).reshape(2, HD, N).transpose(0, 2, 1)  # [2, N, HD]
        o = o / l[:, :, None]
        o[:, 0, :] = 0.0
        out[0, 2 * c : 2 * c + 2] = o
    return out
